# revision 2
# baseline (speedup 1.0000x reference)
"""Trainium2 Bass kernel for nn_FB_Seg_90950227460831 (gnn_message_passing).

Strategy
--------
The reference gathers per-point features from three [64, 512, 512] maps and
runs a small MLP on each of 120000 points per batch.  Random 4-byte gathers
from HBM are hopeless on TRN2 DMA; instead we run the MLP *densely* over every
pixel (only ~2.2x the point FLOPs, and FLOPs are nearly free here), streaming
each map exactly once from HBM -- the memory roofline for this problem.

Sharding: core = (batch b, row-quadrant q).  8 cores each process a
[64ch, 128rows, 512cols] strip of the three maps (50.3 MB streamed / core).

Device pipeline per 512-pixel group (bf16 matmuls, fp32 PSUM, exact-erf gelu
and sigmoid on the ACT LUT engine):
  layer1 (fused):  z1 = [p0;p1] @ (w_lin @ w1[:64]) + fl @ w1[64:] + b1'
                   (exact algebraic fusion of the pc = cat(f0,f1)@w_lin+b_lin
                   linear layer into w1 -- no intermediate pc tensor)
  h1 = gelu(z1); h2 = gelu(h1@w2+b2); h3 = gelu(h2@w3+b3)
  s  = sigmoid(h3@w4+b4)    -> dense per-pixel score map [128, 512] per core
Groups are packed across PSUM partitions with block-diagonal weights so every
ACT op covers 128 partitions (ACT cost is free-dim-only, so partition packing
is the difference between 92us and 460us of ACT time per core).

Host side does only sharding / index bookkeeping: slicing the maps, packing
tiny fused weights, and the final pointwise take s_dense[r, c] per point while
unsharding (pure index manipulation; all FLOPs and bulk data movement are
on-device).
"""

import numpy as np
import ml_dtypes

from contextlib import ExitStack

import concourse.bass as bass
import concourse.tile as tile
from concourse import bacc, mybir
from concourse.bass_utils import run_bass_kernel_spmd

BF16 = mybir.dt.bfloat16
F32 = mybir.dt.float32
AFT = mybir.ActivationFunctionType

B, C, H, W = 2, 64, 512, 512
N_CORES = 8
QROWS = H // 4                  # 128 image rows per core strip
PX = QROWS * W                  # 65536 pixels per core
SC_PX = 8192                    # pixels per superchunk (one DMA load)
N_SC = PX // SC_PX              # 8 superchunks
GRP = 512                       # pixels per matmul group (one PSUM bank)

_COMPILED = {}                  # cache the compiled Bass program per process


def _build_program():
    nc = bacc.Bacc("TRN2", target_bir_lowering=False, debug=False,
                   num_devices=N_CORES)

    p0_d = nc.dram_tensor("p0", [C, PX], F32, kind="ExternalInput")
    p1_d = nc.dram_tensor("p1", [C, PX], F32, kind="ExternalInput")
    fl_d = nc.dram_tensor("fl", [C, PX], F32, kind="ExternalInput")
    wft_d = nc.dram_tensor("wft", [128, 64], BF16, kind="ExternalInput")
    wfb_d = nc.dram_tensor("wfb", [64, 64], BF16, kind="ExternalInput")
    w2b_d = nc.dram_tensor("w2b", [128, 64], BF16, kind="ExternalInput")
    w3b_d = nc.dram_tensor("w3b", [128, 64], BF16, kind="ExternalInput")
    w4b_d = nc.dram_tensor("w4b", [128, 8], BF16, kind="ExternalInput")
    b1_d = nc.dram_tensor("b1p", [128, 1], F32, kind="ExternalInput")
    b2_d = nc.dram_tensor("b2p", [128, 1], F32, kind="ExternalInput")
    b3_d = nc.dram_tensor("b3p", [128, 1], F32, kind="ExternalInput")
    b4_d = nc.dram_tensor("b4p", [8, 1], F32, kind="ExternalInput")

    out_d = nc.dram_tensor("s_dense", [QROWS, W], F32, kind="ExternalOutput")

    with tile.TileContext(nc) as tc:
        with ExitStack() as ctx:
            wpool = ctx.enter_context(tc.tile_pool(name="weights", bufs=1))
            xpool = ctx.enter_context(tc.tile_pool(name="x", bufs=2))
            hpool = ctx.enter_context(tc.tile_pool(name="h", bufs=6))
            spool = ctx.enter_context(tc.tile_pool(name="s", bufs=3))
            ps1p = ctx.enter_context(tc.tile_pool(name="ps1", bufs=2, space="PSUM"))
            ps2p = ctx.enter_context(tc.tile_pool(name="ps2", bufs=2, space="PSUM"))
            ps3p = ctx.enter_context(tc.tile_pool(name="ps3", bufs=2, space="PSUM"))
            ps4p = ctx.enter_context(tc.tile_pool(name="ps4", bufs=2, space="PSUM"))

            wft = wpool.tile([128, 64], BF16)
            nc.sync.dma_start(wft[:], wft_d.ap()[:])
            wfb = wpool.tile([64, 64], BF16)
            nc.sync.dma_start(wfb[:], wfb_d.ap()[:])
            w2b = wpool.tile([128, 64], BF16)
            nc.sync.dma_start(w2b[:], w2b_d.ap()[:])
            w3b = wpool.tile([128, 64], BF16)
            nc.sync.dma_start(w3b[:], w3b_d.ap()[:])
            w4b = wpool.tile([128, 8], BF16)
            nc.sync.dma_start(w4b[:], w4b_d.ap()[:])
            b1p = wpool.tile([128, 1], F32)
            nc.sync.dma_start(b1p[:], b1_d.ap()[:])
            b2p = wpool.tile([128, 1], F32)
            nc.sync.dma_start(b2p[:], b2_d.ap()[:])
            b3p = wpool.tile([128, 1], F32)
            nc.sync.dma_start(b3p[:], b3_d.ap()[:])
            b4p = wpool.tile([8, 1], F32)
            nc.sync.dma_start(b4p[:], b4_d.ap()[:])

            for sc in range(N_SC):
                px0 = sc * SC_PX
                # load + cast f32->bf16 during DMA (SWDGE)
                xb = xpool.tile([128, SC_PX], BF16, tag="xb")
                nc.gpsimd.dma_start(xb[0:64, :], p0_d.ap()[:, px0:px0 + SC_PX])
                nc.gpsimd.dma_start(xb[64:128, :], p1_d.ap()[:, px0:px0 + SC_PX])
                flb = xpool.tile([64, SC_PX], BF16, tag="flb")
                nc.gpsimd.dma_start(flb[:], fl_d.ap()[:, px0:px0 + SC_PX])

                for okta in range(2):           # 4096 px
                    ps3t = ps3p.tile([128, GRP], F32, space="PSUM")
                    for half in range(2):       # 2048 px
                        ps2t = ps2p.tile([128, GRP], F32, space="PSUM")
                        for pair in range(2):   # 1024 px
                            ps1t = ps1p.tile([128, GRP], F32, space="PSUM")
                            for g01 in range(2):    # 512 px
                                off = ((okta * 2 + half) * 2 + pair) * 2 + g01
                                cs = slice(off * GRP, (off + 1) * GRP)
                                tp = (0, 64) if g01 else None
                                po = slice(g01 * 64, (g01 + 1) * 64)
                                nc.tensor.matmul(ps1t[po, :], wft[:], xb[:, cs],
                                                 start=True, stop=False,
                                                 tile_position=tp)
                                nc.tensor.matmul(ps1t[po, :], wfb[:], flb[:, cs],
                                                 start=False, stop=True,
                                                 tile_position=tp)
                            h1t = hpool.tile([128, GRP], BF16, tag="h1")
                            nc.scalar.activation(h1t[:], ps1t[:], AFT.Gelu,
                                                 bias=b1p[:], scale=1.0)
                            tp = (0, 64) if pair else None
                            po = slice(pair * 64, (pair + 1) * 64)
                            nc.tensor.matmul(ps2t[po, :], w2b[:], h1t[:],
                                             start=True, stop=True,
                                             tile_position=tp)
                        h2t = hpool.tile([128, GRP], BF16, tag="h2")
                        nc.scalar.activation(h2t[:], ps2t[:], AFT.Gelu,
                                             bias=b2p[:], scale=1.0)
                        tp = (0, 64) if half else None
                        po = slice(half * 64, (half + 1) * 64)
                        nc.tensor.matmul(ps3t[po, :], w3b[:], h2t[:],
                                         start=True, stop=True,
                                         tile_position=tp)
                    h3t = hpool.tile([128, GRP], BF16, tag="h3")
                    nc.scalar.activation(h3t[:], ps3t[:], AFT.Gelu,
                                         bias=b3p[:], scale=1.0)
                    ps4t = ps4p.tile([8, GRP], F32, space="PSUM")
                    nc.tensor.matmul(ps4t[:], w4b[:], h3t[:],
                                     start=True, stop=True)
                    st = spool.tile([8, GRP], F32, tag="st")
                    nc.scalar.activation(st[:], ps4t[:], AFT.Sigmoid,
                                         bias=b4p[:], scale=1.0)
                    o = sc * 2 + okta           # global okta index [0, 16)
                    nc.sync.dma_start(out_d.ap()[8 * o: 8 * o + 8, :], st[:])

    nc.compile()
    return nc


def kernel(pc0_map, pc1_map, flow_map, lidar_coords, radar_coords,
           w_lin, b_lin, w1, b1, w2, b2, w3, b3, w4, b4):
    f64 = np.float64
    bf = ml_dtypes.bfloat16

    # --- fused / packed weights (tiny; exact linear algebra in float64) ---
    w1a = np.asarray(w1, f64)[:64, :]            # pc -> h1 part
    w1b = np.asarray(w1, f64)[64:, :]            # flow -> h1 part
    wft = (np.asarray(w_lin, f64) @ w1a).astype(bf)             # [128, 64]
    wfb = np.ascontiguousarray(w1b).astype(bf)                  # [64, 64]
    b1f = (np.asarray(b_lin, f64) @ w1a + np.asarray(b1, f64)).astype(np.float32)
    b1p = np.tile(b1f, 2)[:, None].astype(np.float32)           # [128, 1]

    w2n = np.asarray(w2, np.float32)
    w2b = np.zeros((128, 64), dtype=bf)
    w2b[0:64, 0:32] = w2n.astype(bf)
    w2b[64:128, 32:64] = w2n.astype(bf)
    b2p = np.tile(np.asarray(b2, np.float32), 4)[:, None].astype(np.float32)

    w3n = np.asarray(w3, np.float32)
    w3b = np.zeros((128, 64), dtype=bf)
    for v in range(4):
        w3b[32 * v:32 * v + 32, 16 * v:16 * v + 16] = w3n.astype(bf)
    b3p = np.tile(np.asarray(b3, np.float32), 8)[:, None].astype(np.float32)

    w4n = np.asarray(w4, np.float32)
    w4b = np.zeros((128, 8), dtype=bf)
    for u in range(8):
        w4b[16 * u:16 * u + 16, u:u + 1] = w4n.astype(bf)
    b4s = float(np.asarray(b4, np.float64).reshape(-1)[0])

    key = "prog"
    if key not in _COMPILED:
        _COMPILED[key] = _build_program()
    nc = _COMPILED[key]

    # --- shard: core = 4*b + q ; strip = image rows [128q, 128q+128) ---
    in_maps = []
    for b in range(B):
        for q in range(4):
            r0 = q * QROWS
            sl = np.s_[b, :, r0:r0 + QROWS, :]
            in_maps.append({
                "p0": np.ascontiguousarray(pc0_map[sl]).reshape(C, PX),
                "p1": np.ascontiguousarray(pc1_map[sl]).reshape(C, PX),
                "fl": np.ascontiguousarray(flow_map[sl]).reshape(C, PX),
                "wft": wft, "wfb": wfb, "w2b": w2b, "w3b": w3b, "w4b": w4b,
                "b1p": b1p, "b2p": b2p, "b3p": b3p,
                "b4p": np.full((8, 1), b4s, dtype=np.float32),
            })

    res = run_bass_kernel_spmd(nc, in_maps, core_ids=list(range(N_CORES)),
                               trace=False)
    # dense per-pixel score maps, one [128, 512] strip per core
    s_dense = np.stack(
        [np.stack([res.results[4 * b + q]["s_dense"] for q in range(4)], axis=0)
         for b in range(B)], axis=0)             # [B, 4, 128, 512]
    s_dense = s_dense.reshape(B, H, W)

    # --- unshard: pointwise take per (batch, point) ---
    out = np.empty((B, lidar_coords.shape[1] + radar_coords.shape[1]),
                   dtype=np.float32)
    for b in range(B):
        coords = np.concatenate([lidar_coords[b], radar_coords[b]], axis=0)
        r = coords[:, 1].astype(np.int64)
        c = coords[:, 2].astype(np.int64)
        out[b] = s_dense[b, r, c]
    return out


# revision 4
# speedup vs baseline: 1.5106x; 1.5106x over previous
"""Trainium2 Bass kernel for nn_FB_Seg_90950227460831 (gnn_message_passing).

Strategy
--------
The reference gathers per-point features from three [64, 512, 512] maps and
runs a small MLP on each of 120000 points per batch.  Random 4-byte gathers
from HBM are hopeless on TRN2 DMA; instead we run the MLP *densely* over every
pixel (only ~2.2x the point FLOPs, and FLOPs are nearly free here), streaming
each map exactly once from HBM -- the memory roofline for this problem.

Sharding: core = (batch b, row-quadrant q).  8 cores each process a
[64ch, 128rows, 512cols] strip of the three maps (50.3 MB streamed / core).

Device pipeline per 512-pixel group (bf16 matmuls, fp32 PSUM, exact-erf gelu
and sigmoid on the ACT LUT engine):
  layer1 (fused):  z1 = [p0;p1] @ (w_lin @ w1[:64]) + fl @ w1[64:] + b1'
                   (exact algebraic fusion of the pc = cat(f0,f1)@w_lin+b_lin
                   linear layer into w1 -- no intermediate pc tensor)
  h1 = gelu(z1); h2 = gelu(h1@w2+b2); h3 = gelu(h2@w3+b3)
  s  = sigmoid(h3@w4+b4)    -> dense per-pixel score map [128, 512] per core
Groups are packed across PSUM partitions with block-diagonal weights so every
ACT op covers 128 partitions (ACT cost is free-dim-only, so partition packing
is the difference between 92us and 460us of ACT time per core).

Host side does only sharding / index bookkeeping: slicing the maps, packing
tiny fused weights, and the final pointwise take s_dense[r, c] per point while
unsharding (pure index manipulation; all FLOPs and bulk data movement are
on-device).
"""

import numpy as np
import ml_dtypes

from contextlib import ExitStack

import concourse.bass as bass
import concourse.tile as tile
from concourse import bacc, mybir
from concourse.bass_utils import run_bass_kernel_spmd

BF16 = mybir.dt.bfloat16
F32 = mybir.dt.float32
AFT = mybir.ActivationFunctionType

B, C, H, W = 2, 64, 512, 512
N_CORES = 8
QROWS = H // 4                  # 128 image rows per core strip
PX = QROWS * W                  # 65536 pixels per core
SC_PX = 8192                    # pixels per superchunk (one DMA load)
N_SC = PX // SC_PX              # 8 superchunks
GRP = 512                       # pixels per matmul group (one PSUM bank)

_COMPILED = {}                  # cache the compiled Bass program per process


def _build_program():
    nc = bacc.Bacc("TRN2", target_bir_lowering=False, debug=False,
                   num_devices=N_CORES)

    p0_d = nc.dram_tensor("p0", [C, PX], F32, kind="ExternalInput")
    p1_d = nc.dram_tensor("p1", [C, PX], F32, kind="ExternalInput")
    fl_d = nc.dram_tensor("fl", [C, PX], F32, kind="ExternalInput")
    wft_d = nc.dram_tensor("wft", [128, 64], BF16, kind="ExternalInput")
    wfb_d = nc.dram_tensor("wfb", [64, 64], BF16, kind="ExternalInput")
    w2b_d = nc.dram_tensor("w2b", [128, 64], BF16, kind="ExternalInput")
    w3b_d = nc.dram_tensor("w3b", [128, 64], BF16, kind="ExternalInput")
    w4b_d = nc.dram_tensor("w4b", [128, 8], BF16, kind="ExternalInput")
    b1_d = nc.dram_tensor("b1p", [128, 1], F32, kind="ExternalInput")
    b2_d = nc.dram_tensor("b2p", [128, 1], F32, kind="ExternalInput")
    b3_d = nc.dram_tensor("b3p", [128, 1], F32, kind="ExternalInput")
    b4_d = nc.dram_tensor("b4p", [8, 1], F32, kind="ExternalInput")

    out_d = nc.dram_tensor("s_dense", [QROWS, W], F32, kind="ExternalOutput")

    with tile.TileContext(nc) as tc:
        with ExitStack() as ctx:
            wpool = ctx.enter_context(tc.tile_pool(name="weights", bufs=1))
            xpool = ctx.enter_context(tc.tile_pool(name="x", bufs=2))
            hpool = ctx.enter_context(tc.tile_pool(name="h", bufs=10))
            spool = ctx.enter_context(tc.tile_pool(name="s", bufs=3))
            psp = ctx.enter_context(tc.tile_pool(name="ps", bufs=8, space="PSUM"))

            wft = wpool.tile([128, 64], BF16)
            nc.sync.dma_start(wft[:], wft_d.ap()[:])
            wfb = wpool.tile([64, 64], BF16)
            nc.sync.dma_start(wfb[:], wfb_d.ap()[:])
            w2b = wpool.tile([128, 64], BF16)
            nc.sync.dma_start(w2b[:], w2b_d.ap()[:])
            w3b = wpool.tile([128, 64], BF16)
            nc.sync.dma_start(w3b[:], w3b_d.ap()[:])
            w4b = wpool.tile([128, 8], BF16)
            nc.sync.dma_start(w4b[:], w4b_d.ap()[:])
            b1p = wpool.tile([128, 1], F32)
            nc.sync.dma_start(b1p[:], b1_d.ap()[:])
            b2p = wpool.tile([128, 1], F32)
            nc.sync.dma_start(b2p[:], b2_d.ap()[:])
            b3p = wpool.tile([128, 1], F32)
            nc.sync.dma_start(b3p[:], b3_d.ap()[:])
            b4p = wpool.tile([8, 1], F32)
            nc.sync.dma_start(b4p[:], b4_d.ap()[:])

            zpool = ctx.enter_context(tc.tile_pool(name="z", bufs=16))
            sig_inputs = []
            for sc in range(N_SC):
                px0 = sc * SC_PX
                # load + cast f32->bf16 during DMA (SWDGE)
                xb = xpool.tile([128, SC_PX], BF16, tag="xb")
                nc.gpsimd.dma_start(xb[0:64, :], p0_d.ap()[:, px0:px0 + SC_PX])
                nc.gpsimd.dma_start(xb[64:128, :], p1_d.ap()[:, px0:px0 + SC_PX])
                flb = xpool.tile([64, SC_PX], BF16, tag="flb")
                nc.gpsimd.dma_start(flb[:], fl_d.ap()[:, px0:px0 + SC_PX])

                NG = SC_PX // GRP          # 16 groups per superchunk
                gs = lambda g: slice(g * GRP, (g + 1) * GRP)
                half = lambda i: slice((i % 2) * 64, (i % 2) * 64 + 64)
                tpos = lambda i: (0, 64) if (i % 2) else None

                # phase 1a/1b: fused layer-1, same-weight streaks
                ps1 = [psp.tile([128, GRP], F32, space="PSUM", tag="ps",
                                name=f"ps1_{sc}_{_i}")
                       for _i in range(NG // 2)]
                for g in range(NG):
                    nc.tensor.matmul(ps1[g // 2][half(g), :], wft[:], xb[:, gs(g)],
                                     start=True, stop=False, tile_position=tpos(g))
                for g in range(NG):
                    nc.tensor.matmul(ps1[g // 2][half(g), :], wfb[:], flb[:, gs(g)],
                                     start=False, stop=True, tile_position=tpos(g))
                h1 = []
                for i in range(NG // 2):
                    t = hpool.tile([128, GRP], BF16, tag="h1")
                    nc.scalar.activation(t[:], ps1[i][:], AFT.Gelu,
                                         bias=b1p[:], scale=1.0)
                    h1.append(t)
                # phase 2
                ps2 = [psp.tile([128, GRP], F32, space="PSUM", tag="ps",
                                name=f"ps2_{sc}_{_i}")
                       for _i in range(NG // 4)]
                for i in range(NG // 2):
                    nc.tensor.matmul(ps2[i // 2][half(i), :], w2b[:], h1[i][:],
                                     start=True, stop=True, tile_position=tpos(i))
                h2 = []
                for j in range(NG // 4):
                    t = hpool.tile([128, GRP], BF16, tag="h2")
                    nc.scalar.activation(t[:], ps2[j][:], AFT.Gelu,
                                         bias=b2p[:], scale=1.0)
                    h2.append(t)
                # phase 3
                ps3 = [psp.tile([128, GRP], F32, space="PSUM", tag="ps",
                                name=f"ps3_{sc}_{_i}")
                       for _i in range(NG // 8)]
                for j in range(NG // 4):
                    nc.tensor.matmul(ps3[j // 2][half(j), :], w3b[:], h2[j][:],
                                     start=True, stop=True, tile_position=tpos(j))
                h3 = []
                for k in range(NG // 8):
                    t = hpool.tile([128, GRP], BF16, tag="h3")
                    nc.scalar.activation(t[:], ps3[k][:], AFT.Gelu,
                                         bias=b3p[:], scale=1.0)
                    h3.append(t)
                # phase 4: final 16->1 layer, then park z in SBUF (DVE copy)
                for k in range(NG // 8):
                    ps4t = psp.tile([8, GRP], F32, space="PSUM", tag="ps")
                    nc.tensor.matmul(ps4t[:], w4b[:], h3[k][:],
                                     start=True, stop=True)
                    zt = zpool.tile([8, GRP], F32, tag="z")
                    nc.vector.tensor_copy(zt[:], ps4t[:])
                    sig_inputs.append((sc * 2 + k, zt))

            # all sigmoids at the end: one ACT table switch instead of 32
            for o, zt in sig_inputs:
                st = spool.tile([8, GRP], F32, tag="st")
                nc.scalar.activation(st[:], zt[:], AFT.Sigmoid,
                                     bias=b4p[:], scale=1.0)
                nc.sync.dma_start(out_d.ap()[8 * o: 8 * o + 8, :], st[:])

    nc.compile()
    return nc


def kernel(pc0_map, pc1_map, flow_map, lidar_coords, radar_coords,
           w_lin, b_lin, w1, b1, w2, b2, w3, b3, w4, b4):
    f64 = np.float64
    bf = ml_dtypes.bfloat16

    # --- fused / packed weights (tiny; exact linear algebra in float64) ---
    w1a = np.asarray(w1, f64)[:64, :]            # pc -> h1 part
    w1b = np.asarray(w1, f64)[64:, :]            # flow -> h1 part
    wft = (np.asarray(w_lin, f64) @ w1a).astype(bf)             # [128, 64]
    wfb = np.ascontiguousarray(w1b).astype(bf)                  # [64, 64]
    b1f = (np.asarray(b_lin, f64) @ w1a + np.asarray(b1, f64)).astype(np.float32)
    b1p = np.tile(b1f, 2)[:, None].astype(np.float32)           # [128, 1]

    w2n = np.asarray(w2, np.float32)
    w2b = np.zeros((128, 64), dtype=bf)
    w2b[0:64, 0:32] = w2n.astype(bf)
    w2b[64:128, 32:64] = w2n.astype(bf)
    b2p = np.tile(np.asarray(b2, np.float32), 4)[:, None].astype(np.float32)

    w3n = np.asarray(w3, np.float32)
    w3b = np.zeros((128, 64), dtype=bf)
    for v in range(4):
        w3b[32 * v:32 * v + 32, 16 * v:16 * v + 16] = w3n.astype(bf)
    b3p = np.tile(np.asarray(b3, np.float32), 8)[:, None].astype(np.float32)

    w4n = np.asarray(w4, np.float32)
    w4b = np.zeros((128, 8), dtype=bf)
    for u in range(8):
        w4b[16 * u:16 * u + 16, u:u + 1] = w4n.astype(bf)
    b4s = float(np.asarray(b4, np.float64).reshape(-1)[0])

    key = "prog"
    if key not in _COMPILED:
        _COMPILED[key] = _build_program()
    nc = _COMPILED[key]

    # --- shard: core = 4*b + q ; strip = image rows [128q, 128q+128) ---
    in_maps = []
    for b in range(B):
        for q in range(4):
            r0 = q * QROWS
            sl = np.s_[b, :, r0:r0 + QROWS, :]
            in_maps.append({
                "p0": np.ascontiguousarray(pc0_map[sl]).reshape(C, PX),
                "p1": np.ascontiguousarray(pc1_map[sl]).reshape(C, PX),
                "fl": np.ascontiguousarray(flow_map[sl]).reshape(C, PX),
                "wft": wft, "wfb": wfb, "w2b": w2b, "w3b": w3b, "w4b": w4b,
                "b1p": b1p, "b2p": b2p, "b3p": b3p,
                "b4p": np.full((8, 1), b4s, dtype=np.float32),
            })

    res = run_bass_kernel_spmd(nc, in_maps, core_ids=list(range(N_CORES)),
                               trace=False)
    # dense per-pixel score maps, one [128, 512] strip per core
    s_dense = np.stack(
        [np.stack([res.results[4 * b + q]["s_dense"] for q in range(4)], axis=0)
         for b in range(B)], axis=0)             # [B, 4, 128, 512]
    s_dense = s_dense.reshape(B, H, W)

    # --- unshard: pointwise take per (batch, point) ---
    out = np.empty((B, lidar_coords.shape[1] + radar_coords.shape[1]),
                   dtype=np.float32)
    for b in range(B):
        coords = np.concatenate([lidar_coords[b], radar_coords[b]], axis=0)
        r = coords[:, 1].astype(np.int64)
        c = coords[:, 2].astype(np.int64)
        out[b] = s_dense[b, r, c]
    return out
